# revision 28
# baseline (speedup 1.0000x reference)
"""Trainium2 Bass kernel for nn_Attention_35021163332119.

Full multi-head attention: qkv = x @ w_qkv; RoPE(q, k); softmax(q k^T / sqrt(dh)) v;
out = heads @ w_out + b_out.  B=2, N=2048, DIM=1024, H=16, DH=64.

Sharding: 8 cores = (batch b in {0,1}) x (head-group g in {0..3} of 4 heads).
Each core computes its 4 heads end-to-end plus the partial output projection
for its head-group's rows of w_out; the host sums the 4 partials per batch
and adds b_out.

v3: the attention inner loop is Activation-engine-bound (2 exps of
[128,1024] per j-tile-pair ~ 2016ns vs ~1344ns of PE matmul), so the
schedule starts the exp stream as early as possible and hides every other
PE job inside the Act-paced window:
- pair-0 QKV runs k-OUTER so matmuls start when the first xT k-tile lands;
  RoPE is applied per 512-column chunk right behind each QKV pass.
- attention(0,0) starts as soon as pair-0 RoPE and half of v are done
  (~38us); the remaining v tiles, the ENTIRE pair-1 QKV/RoPE, and the
  output projection are emitted as small "filler" jobs woven between the
  score/PV groups of later blocks, filling PE slack without stalling exp.
- All PSUM->SBUF copies are on DVE; the scalar engine runs exp only.
- q/k tiles are bf16 (halves SBUF + speeds DVE rope); partial output is
  written bf16.
"""

import numpy as np

B, N, DIM, H, DH = 2, 2048, 1024, 16, 64
ROPE_BASE = 10000.0
SCALE = DH ** -0.5
N_CORES = 8
G = 4                 # heads per core
KT = DIM // 128       # contraction tiles
NT = N // 128         # sequence tiles

_cache = {}


def _rope_tables():
    inv_freq = (1.0 / (ROPE_BASE ** (np.arange(0, DH, 2, dtype=np.float32) / DH)))
    t = np.arange(N, dtype=np.float32)
    freqs = t[:, None] * inv_freq[None, :]          # [N, DH/2]
    freqs = np.repeat(freqs, 2, axis=-1)            # [N, DH] interleaved
    cosT = np.cos(freqs).T.astype(np.float32)       # [DH, N]
    sinT = np.sin(freqs).T.astype(np.float32)
    cos2 = np.concatenate([cosT, cosT], axis=0)     # [128, N] two heads stacked
    sin2 = np.concatenate([sinT, sinT], axis=0)
    return np.ascontiguousarray(cos2), np.ascontiguousarray(sin2)


def _p2t():
    # rot = P2 @ qT with P2 = blockdiag(P, P), P[2t, 2t+1] = -1, P[2t+1, 2t] = 1
    # matmul computes lhsT.T @ rhs, so pass P2.T
    p = np.zeros((DH, DH), dtype=np.float32)
    for t in range(DH // 2):
        p[2 * t, 2 * t + 1] = -1.0
        p[2 * t + 1, 2 * t] = 1.0
    p2 = np.zeros((128, 128), dtype=np.float32)
    p2[:DH, :DH] = p
    p2[DH:, DH:] = p
    return np.ascontiguousarray(p2.T)


def _build():
    if "nc" in _cache:
        return _cache["nc"]

    import concourse.mybir as mybir
    import concourse.tile as tile
    from concourse import bacc

    F32 = mybir.dt.float32
    BF16 = mybir.dt.bfloat16
    EXP = mybir.ActivationFunctionType.Exp

    nc = bacc.Bacc("TRN2", target_bir_lowering=False, debug=False)
    xT_d = nc.dram_tensor("xT", [DIM, N], BF16, kind="ExternalInput")
    w_d = nc.dram_tensor("wqkv", [DIM, 768], BF16, kind="ExternalInput")
    wout_d = nc.dram_tensor("wout", [G * DH, DIM], BF16, kind="ExternalInput")
    cos_d = nc.dram_tensor("cos2", [128, N], BF16, kind="ExternalInput")
    sin_d = nc.dram_tensor("sin2", [128, N], BF16, kind="ExternalInput")
    p2t_d = nc.dram_tensor("p2t", [128, 128], BF16, kind="ExternalInput")
    part_d = nc.dram_tensor("part", [N, DIM], BF16, kind="ExternalOutput")

    with tile.TileContext(nc) as tc:
        with tc.tile_pool(name="persist", bufs=1) as persist, \
             tc.tile_pool(name="att", bufs=5) as att, \
             tc.tile_pool(name="norm_w", bufs=2) as norm_w, \
             tc.tile_pool(name="outp", bufs=3) as outp, \
             tc.tile_pool(name="xph", bufs=1) as xph, \
             tc.tile_pool(name="rope_w", bufs=2) as rope_w, \
             tc.tile_pool(name="ps", bufs=2, space="PSUM") as ps, \
             tc.tile_pool(name="pso", bufs=2, space="PSUM") as pso, \
             tc.tile_pool(name="psf", bufs=2, space="PSUM") as psf:

            # ---- persistent tiles ----
            qk_sb = [persist.tile([128, N], BF16, tag=f"qk{m}", name=f"qk{m}")
                     for m in range(4)]          # q01T, q23T, k01T, k23T
            v_aug = persist.tile([128, NT, G, DH + 1], BF16, tag="vaug")
            wout_sb = [persist.tile([128, DIM], BF16, tag=f"wo{kk}", name=f"wo{kk}")
                       for kk in range(2)]
            outT = [persist.tile([128, N], BF16, tag=f"outT{p}", name=f"outT{p}")
                    for p in range(2)]

            # ---- phase-1 tiles ----
            xT = [xph.tile([128, N], BF16, tag=f"xT{k}", name=f"xT{k}")
                  for k in range(KT)]
            wqkv = [xph.tile([128, 768], BF16, tag=f"wq{k}", name=f"wq{k}")
                    for k in range(KT)]
            cos2 = xph.tile([128, N], BF16, tag="cos2")
            sin2 = xph.tile([128, N], BF16, tag="sin2")
            p2t = xph.tile([128, 128], BF16, tag="p2t")
            ones_col = xph.tile([128, NT, G, 1], F32, tag="ones")

            # ---- input DMA in consumption order ----
            xT_r = xT_d.ap().rearrange("(t p) n -> t p n", p=128)
            w_r = w_d.ap().rearrange("(t p) m -> t p m", p=128)
            for k in range(KT):
                nc.sync.dma_start(out=xT[k][:, 0:1024], in_=xT_r[k][:, 0:1024])
                nc.sync.dma_start(out=wqkv[k], in_=w_r[k])
            nc.sync.dma_start(out=cos2[:, 0:1024], in_=cos_d.ap()[:, 0:1024])
            nc.sync.dma_start(out=sin2[:, 0:1024], in_=sin_d.ap()[:, 0:1024])
            nc.sync.dma_start(out=p2t, in_=p2t_d.ap())
            for k in range(KT):
                nc.sync.dma_start(out=xT[k][:, 1024:2048],
                                  in_=xT_r[k][:, 1024:2048])
            nc.sync.dma_start(out=cos2[:, 1024:2048],
                              in_=cos_d.ap()[:, 1024:2048])
            nc.sync.dma_start(out=sin2[:, 1024:2048],
                              in_=sin_d.ap()[:, 1024:2048])
            for kk in range(2):
                nc.sync.dma_start(
                    out=wout_sb[kk],
                    in_=wout_d.ap().rearrange("(t p) m -> t p m", p=128)[kk])
            nc.vector.memset(ones_col, 1.0)

            def qk_pass0(cpass):
                """k-outer accumulation of pair-0 q/k tiles for one 1024-wide
                column chunk; consumes each xT k-tile as its DMA lands."""
                csl = slice(cpass * 1024, (cpass + 1) * 1024)
                psq = [ps.tile([128, 1024], F32, tag="s", name=f"qk_ps{m2}")
                       for m2 in range(2)]
                for k in range(KT):
                    for m2 in range(2):
                        m = 0 if m2 == 0 else 2
                        for half in range(2):
                            hsl = slice(half * 512, (half + 1) * 512)
                            xsl = slice(cpass * 1024 + half * 512,
                                        cpass * 1024 + (half + 1) * 512)
                            nc.tensor.matmul(
                                psq[m2][:, hsl],
                                wqkv[k][:, m * 128:(m + 1) * 128],
                                xT[k][:, xsl],
                                start=(k == 0), stop=(k == KT - 1))
                for m2 in range(2):
                    m = 0 if m2 == 0 else 2
                    nc.vector.tensor_copy(qk_sb[m][:, csl], psq[m2])

            def rope_chunk(m, c):
                """RoPE on one 512-col chunk of qk_sb[m]: rotate-half via the
                PE permutation matmul, then combine with cos/sin on DVE."""
                csl = slice(c * 512, (c + 1) * 512)
                rps = psf.tile([128, 512], F32, tag="f", name="rot")
                nc.tensor.matmul(rps, p2t, qk_sb[m][:, csl],
                                 start=True, stop=True)
                tmp = rope_w.tile([128, 512], BF16, tag="rtmp")
                nc.vector.tensor_mul(tmp, rps, sin2[:, csl])
                nc.vector.tensor_mul(qk_sb[m][:, csl], qk_sb[m][:, csl],
                                     cos2[:, csl])
                nc.vector.tensor_add(qk_sb[m][:, csl], qk_sb[m][:, csl], tmp)

            def v_group(tn):
                mm_ps = psf.tile([128, 512], F32, tag="f", name="mm_v")
                for k in range(KT):
                    nc.tensor.matmul(
                        mm_ps[:, 0:G * DH],
                        xT[k][:, tn * 128:(tn + 1) * 128],
                        wqkv[k][:, 512:768],
                        start=(k == 0), stop=(k == KT - 1))
                nc.scalar.copy(
                    v_aug[:, tn, :, 0:DH],
                    mm_ps[:, 0:G * DH].rearrange("p (h d) -> p h d", h=G))
                nc.vector.tensor_copy(v_aug[:, tn, :, DH:DH + 1],
                                      ones_col[:, tn])

            # pair-1 QKV as filler jobs: k-QUARTER per (m2, 512-col chunk) so
            # each job holds a PSUM buf only ~1us; halves are combined via an
            # SBUF staging tile and a DVE add.
            p1_stage = {}

            def pass1_q(m2, chunk, q):
                m = 1 if m2 == 0 else 3
                csl = slice(chunk * 512, (chunk + 1) * 512)
                t = psf.tile([128, 512], F32, tag="f", name="p1")
                for k in range(q * 4, q * 4 + 4):
                    nc.tensor.matmul(
                        t,
                        wqkv[k][:, m * 128:(m + 1) * 128], xT[k][:, csl],
                        start=(k == q * 4), stop=(k == q * 4 + 3))
                if q == 0:
                    st = rope_w.tile([128, 512], F32, tag="p1st")
                    p1_stage[(m2, chunk)] = st
                    nc.vector.tensor_copy(st, t)
                else:
                    st = p1_stage.pop((m2, chunk))
                    nc.vector.tensor_add(qk_sb[m][:, csl], st, t)

            def normalize(p, iq, o_ps):
                """PV accumulators -> normalized bf16 rows of outT."""
                isl = slice(iq * 512, iq * 512 + 512)
                for hh in range(2):
                    o_sb = norm_w.tile([DH + 1, 512], F32, tag=f"osb{hh}",
                                       name=f"osb{hh}")
                    nc.vector.tensor_copy(o_sb, o_ps[hh])
                    recip0 = norm_w.tile([1, 512], F32, tag=f"r0{hh}",
                                         name=f"r0{hh}")
                    nc.sync.dma_start(out=recip0, in_=o_sb[DH:DH + 1, :])
                    nc.vector.reciprocal_approx_fast(recip0, recip0)
                    bc = norm_w.tile([DH, 512], F32, tag=f"bc{hh}",
                                     name=f"bc{hh}")
                    nc.gpsimd.partition_broadcast(bc, recip0)
                    if hh == 0:
                        nc.vector.tensor_mul(outT[p][0:DH, isl],
                                             o_sb[0:DH, :], bc)
                    else:
                        tmpb = norm_w.tile([DH, 512], BF16, tag="tmpb")
                        nc.vector.tensor_mul(tmpb, o_sb[0:DH, :], bc)
                        nc.sync.dma_start(out=outT[p][DH:2 * DH, isl],
                                          in_=tmpb)

            def emit_pv(p, o_ps, jj, exps):
                for hh in range(2):
                    for half in range(2):
                        j = 2 * jj + half
                        nc.tensor.matmul(
                            o_ps[hh],
                            v_aug[:, j, 2 * p + hh, :],
                            exps[hh][:, half * 512:(half + 1) * 512],
                            start=(j == 0), stop=(j == NT - 1))

            def attention(p, iq, fillers=(), slots=None):
                """Scores -> exp -> PV pipeline for one (pair, i-quarter)
                block; pops filler jobs into PE slack at the given j-slots."""
                o_ps = [pso.tile([DH + 1, 512], F32, tag="o", name=f"o{hh}")
                        for hh in range(2)]
                qT = qk_sb[p]
                kTt = qk_sb[2 + p]
                isl = slice(iq * 512, iq * 512 + 512)
                fillers = list(fillers)
                if slots is None:
                    slots = list(range(len(fillers)))
                fi = 0
                pend_exps = None
                for jj in range(NT // 2):
                    s_ps = [ps.tile([128, 1024], F32, tag="s", name=f"s{hh}")
                            for hh in range(2)]
                    for half in range(2):
                        j = 2 * jj + half
                        jsl = slice(j * 128, (j + 1) * 128)
                        for hh in range(2):
                            hsl = slice(hh * DH, (hh + 1) * DH)
                            # K=64 pair in disjoint PE row groups -> the two
                            # matmuls run concurrently (explicit tile_position;
                            # auto-derive does not engage it)
                            nc.tensor.matmul(
                                s_ps[hh][:, half * 512:(half + 1) * 512],
                                kTt[hsl, jsl], qT[hsl, isl],
                                start=True, stop=True,
                                tile_position=(hh * DH, 0))
                    exps = []
                    for hh in range(2):
                        expT = att.tile([128, 1024], BF16, tag="exp")
                        nc.scalar.activation(expT, s_ps[hh], EXP, scale=SCALE)
                        exps.append(expT)
                    if pend_exps is not None:
                        emit_pv(p, o_ps, jj - 1, pend_exps)
                    pend_exps = exps
                    while fi < len(fillers) and fi < len(slots) \
                            and slots[fi] <= jj:
                        fillers[fi]()
                        fi += 1
                emit_pv(p, o_ps, NT // 2 - 1, pend_exps)
                while fi < len(fillers):
                    fillers[fi]()
                    fi += 1
                normalize(p, iq, o_ps)

            def proj_tile(tn, cp_act=False):
                nsl = slice(tn * 128, (tn + 1) * 128)
                out_sb = outp.tile([128, DIM], BF16, tag="osb")
                for c2 in range(2):
                    c2sl = slice(c2 * 512, (c2 + 1) * 512)
                    f_ps = psf.tile([128, 512], F32, tag="f", name="f_ps")
                    for kk in range(2):
                        nc.tensor.matmul(
                            f_ps,
                            outT[kk][:, nsl], wout_sb[kk][:, c2sl],
                            start=(kk == 0), stop=(kk == 1))
                    if cp_act:
                        nc.scalar.copy(out_sb[:, c2sl], f_ps)
                    else:
                        nc.vector.tensor_copy(out_sb[:, c2sl], f_ps)
                nc.sync.dma_start(
                    out=part_d.ap().rearrange("(t p) m -> t p m", p=128)[tn],
                    in_=out_sb)

            # ---- emission order ----
            # pass0 chunk B needs xT cols 1024:2048, which land ~5us after
            # the first half; v_group(0..7) only reads cols 0:1024, so it
            # fills the PE while that DMA streams.
            qk_pass0(0)
            for m in (0, 2):
                for c in (0, 1):
                    rope_chunk(m, c)
            for tn in range(0, 8):
                v_group(tn)
            qk_pass0(1)
            for m in (0, 2):
                for c in (2, 3):
                    rope_chunk(m, c)

            def F(fn, *a):
                return lambda: fn(*a)

            # filler inventory for the later attention blocks:
            #   8 v groups, 16 pass-1 k-quarters, 8 pair-1 rope chunks,
            #   12 projection tiles.
            # All pair-1 work must complete inside the pair-0 blocks —
            # attention(1,*) reads the roped qk_sb[1]/[3].  proj(iq) lags its
            # attention blocks by >= 2 block-slots (late slots of block iq+1)
            # so it never waits on a normalize chain.
            f_v = [F(v_group, tn) for tn in range(8, 16)]
            f_p1 = [F(pass1_q, m2, c, q)
                    for m2 in range(2) for c in range(4) for q in range(2)]
            f_r1 = [F(rope_chunk, m, c) for m in (1, 3) for c in range(4)]

            attention(0, 0, f_v)
            attention(0, 1, f_p1[0:8])
            attention(0, 2, f_p1[8:16])
            attention(0, 3, f_r1)
            attention(1, 0)
            attention(1, 1, [F(proj_tile, tn) for tn in range(0, 4)],
                      slots=[4, 5, 6, 7])
            attention(1, 2, [F(proj_tile, tn) for tn in range(4, 8)],
                      slots=[4, 5, 6, 7])
            attention(1, 3, [F(proj_tile, tn) for tn in range(8, 10)],
                      slots=[5, 7])
            # proj 10,11 depend only on normalize(1,2): they keep the PE warm
            # while normalize(1,3) drains; 12-15 follow it.
            for tn in range(10, 12):
                proj_tile(tn)
            for tn in range(12, 16):
                proj_tile(tn, cp_act=(tn % 2 == 0))
    nc.compile()
    _cache["nc"] = nc
    return nc


def kernel(x, w_qkv, w_out, b_out, _trace=False):
    import ml_dtypes
    from concourse.bass_utils import run_bass_kernel_spmd

    x = np.asarray(x, dtype=np.float32)
    w_qkv = np.asarray(w_qkv, dtype=np.float32)
    w_out = np.asarray(w_out, dtype=np.float32)
    b_out = np.asarray(b_out, dtype=np.float32)

    cos2, sin2 = _rope_tables()
    p2t = _p2t()

    in_maps = []
    for c in range(N_CORES):
        b, g = divmod(c, G)
        cols = []
        for blk in range(2):                      # q block, k block
            base = blk * H * DH + g * G * DH
            cols.append(w_qkv[:, base:base + G * DH])
        cols.append(w_qkv[:, 2 * H * DH + g * G * DH:
                          2 * H * DH + (g + 1) * G * DH])   # v block
        wqkv_c = np.ascontiguousarray(np.concatenate(cols, axis=1))  # [DIM,768]
        wout_c = np.ascontiguousarray(
            w_out[g * G * DH:(g + 1) * G * DH, :]).astype(ml_dtypes.bfloat16)
        in_maps.append({
            "xT": np.ascontiguousarray(x[b].T).astype(ml_dtypes.bfloat16),
            "wqkv": wqkv_c.astype(ml_dtypes.bfloat16),
            "wout": wout_c,
            "cos2": cos2.astype(ml_dtypes.bfloat16),
            "sin2": sin2.astype(ml_dtypes.bfloat16),
            "p2t": p2t.astype(ml_dtypes.bfloat16),
        })

    nc = _build()
    res = run_bass_kernel_spmd(nc, in_maps, core_ids=list(range(N_CORES)),
                               trace=_trace)
    out = np.empty((B, N, DIM), dtype=np.float32)
    for b in range(B):
        acc = res.results[G * b]["part"].astype(np.float32)
        for g in range(1, G):
            acc += res.results[G * b + g]["part"].astype(np.float32)
        out[b] = acc + b_out
    if _trace:
        kernel.last_results = res
    return out


# revision 34
# speedup vs baseline: 1.1849x; 1.1849x over previous
"""Trainium2 Bass kernel for nn_Attention_35021163332119.

Full multi-head attention: qkv = x @ w_qkv; RoPE(q, k); softmax(q k^T / sqrt(dh)) v;
out = heads @ w_out + b_out.  B=2, N=2048, DIM=1024, H=16, DH=64.

Sharding: 8 cores = (batch b in {0,1}) x (head-group g in {0..3} of 4 heads).
Each core computes its 4 heads end-to-end plus the partial output projection
for its head-group's rows of w_out; the host sums the 4 partials per batch
and adds b_out.

v3: the attention inner loop is Activation-engine-bound (2 exps of
[128,1024] per j-tile-pair ~ 2016ns vs ~1344ns of PE matmul), so the
schedule starts the exp stream as early as possible and hides every other
PE job inside the Act-paced window:
- pair-0 QKV runs k-OUTER so matmuls start when the first xT k-tile lands;
  RoPE is applied per 512-column chunk right behind each QKV pass.
- attention(0,0) starts as soon as pair-0 RoPE and half of v are done
  (~38us); the remaining v tiles, the ENTIRE pair-1 QKV/RoPE, and the
  output projection are emitted as small "filler" jobs woven between the
  score/PV groups of later blocks, filling PE slack without stalling exp.
- All PSUM->SBUF copies are on DVE; the scalar engine runs exp only.
- q/k tiles are bf16 (halves SBUF + speeds DVE rope); partial output is
  written bf16.
"""

import numpy as np

B, N, DIM, H, DH = 2, 2048, 1024, 16, 64
ROPE_BASE = 10000.0
SCALE = DH ** -0.5
N_CORES = 8
G = 4                 # heads per core
KT = DIM // 128       # contraction tiles
NT = N // 128         # sequence tiles

_cache = {}


def _rope_tables():
    inv_freq = (1.0 / (ROPE_BASE ** (np.arange(0, DH, 2, dtype=np.float32) / DH)))
    t = np.arange(N, dtype=np.float32)
    freqs = t[:, None] * inv_freq[None, :]          # [N, DH/2]
    freqs = np.repeat(freqs, 2, axis=-1)            # [N, DH] interleaved
    cosT = np.cos(freqs).T.astype(np.float32)       # [DH, N]
    sinT = np.sin(freqs).T.astype(np.float32)
    cos2 = np.concatenate([cosT, cosT], axis=0)     # [128, N] two heads stacked
    sin2 = np.concatenate([sinT, sinT], axis=0)
    return np.ascontiguousarray(cos2), np.ascontiguousarray(sin2)


def _p2t():
    # rot = P2 @ qT with P2 = blockdiag(P, P), P[2t, 2t+1] = -1, P[2t+1, 2t] = 1
    # matmul computes lhsT.T @ rhs, so pass P2.T
    p = np.zeros((DH, DH), dtype=np.float32)
    for t in range(DH // 2):
        p[2 * t, 2 * t + 1] = -1.0
        p[2 * t + 1, 2 * t] = 1.0
    p2 = np.zeros((128, 128), dtype=np.float32)
    p2[:DH, :DH] = p
    p2[DH:, DH:] = p
    return np.ascontiguousarray(p2.T)


def _build():
    if "nc" in _cache:
        return _cache["nc"]

    import concourse.mybir as mybir
    import concourse.tile as tile
    from concourse import bacc

    F32 = mybir.dt.float32
    BF16 = mybir.dt.bfloat16
    EXP = mybir.ActivationFunctionType.Exp

    nc = bacc.Bacc("TRN2", target_bir_lowering=False, debug=False)
    xT_d = nc.dram_tensor("xT", [DIM, N], BF16, kind="ExternalInput")
    w_d = nc.dram_tensor("wqkv", [DIM, 768], BF16, kind="ExternalInput")
    wout_d = nc.dram_tensor("wout", [G * DH, DIM], BF16, kind="ExternalInput")
    cos_d = nc.dram_tensor("cos2", [128, N], BF16, kind="ExternalInput")
    sin_d = nc.dram_tensor("sin2", [128, N], BF16, kind="ExternalInput")
    p2t_d = nc.dram_tensor("p2t", [128, 128], BF16, kind="ExternalInput")
    part_d = nc.dram_tensor("part", [N, DIM], BF16, kind="ExternalOutput")

    with tile.TileContext(nc) as tc:
        with tc.tile_pool(name="persist", bufs=1) as persist, \
             tc.tile_pool(name="att", bufs=5) as att, \
             tc.tile_pool(name="norm_w", bufs=2) as norm_w, \
             tc.tile_pool(name="outp", bufs=3) as outp, \
             tc.tile_pool(name="xph", bufs=1) as xph, \
             tc.tile_pool(name="rope_w", bufs=2) as rope_w, \
             tc.tile_pool(name="ps", bufs=3, space="PSUM") as ps, \
             tc.tile_pool(name="pso", bufs=2, space="PSUM") as pso:

            # ---- persistent tiles ----
            qk_sb = [persist.tile([128, N], BF16, tag=f"qk{m}", name=f"qk{m}")
                     for m in range(4)]          # q01T, q23T, k01T, k23T
            v_aug = persist.tile([128, NT, G, DH + 1], BF16, tag="vaug")
            wout_sb = [persist.tile([128, DIM], BF16, tag=f"wo{kk}", name=f"wo{kk}")
                       for kk in range(2)]
            outT = [persist.tile([128, N], BF16, tag=f"outT{p}", name=f"outT{p}")
                    for p in range(2)]

            # ---- phase-1 tiles ----
            xT = [xph.tile([128, N], BF16, tag=f"xT{k}", name=f"xT{k}")
                  for k in range(KT)]
            wqkv = [xph.tile([128, 768], BF16, tag=f"wq{k}", name=f"wq{k}")
                    for k in range(KT)]
            cos2 = xph.tile([128, N], BF16, tag="cos2")
            sin2 = xph.tile([128, N], BF16, tag="sin2")
            p2t = xph.tile([128, 128], BF16, tag="p2t")
            ones_col = xph.tile([128, NT, G, 1], F32, tag="ones")

            # ---- input DMA in consumption order ----
            xT_r = xT_d.ap().rearrange("(t p) n -> t p n", p=128)
            w_r = w_d.ap().rearrange("(t p) m -> t p m", p=128)
            for k in range(KT):
                nc.sync.dma_start(out=xT[k][:, 0:1024], in_=xT_r[k][:, 0:1024])
                nc.sync.dma_start(out=wqkv[k], in_=w_r[k])
            nc.sync.dma_start(out=cos2[:, 0:1024], in_=cos_d.ap()[:, 0:1024])
            nc.sync.dma_start(out=sin2[:, 0:1024], in_=sin_d.ap()[:, 0:1024])
            nc.sync.dma_start(out=p2t, in_=p2t_d.ap())
            for k in range(KT):
                nc.sync.dma_start(out=xT[k][:, 1024:2048],
                                  in_=xT_r[k][:, 1024:2048])
            nc.sync.dma_start(out=cos2[:, 1024:2048],
                              in_=cos_d.ap()[:, 1024:2048])
            nc.sync.dma_start(out=sin2[:, 1024:2048],
                              in_=sin_d.ap()[:, 1024:2048])
            for kk in range(2):
                nc.sync.dma_start(
                    out=wout_sb[kk],
                    in_=wout_d.ap().rearrange("(t p) m -> t p m", p=128)[kk])
            nc.vector.memset(ones_col, 1.0)

            def qk_pass0(cpass):
                """k-outer accumulation of pair-0 q/k tiles for one 1024-wide
                column chunk; consumes each xT k-tile as its DMA lands."""
                csl = slice(cpass * 1024, (cpass + 1) * 1024)
                psq = [ps.tile([128, 1024], F32, tag="s", name=f"qk_ps{m2}")
                       for m2 in range(2)]
                for k in range(KT):
                    for m2 in range(2):
                        m = 0 if m2 == 0 else 2
                        for half in range(2):
                            hsl = slice(half * 512, (half + 1) * 512)
                            xsl = slice(cpass * 1024 + half * 512,
                                        cpass * 1024 + (half + 1) * 512)
                            nc.tensor.matmul(
                                psq[m2][:, hsl],
                                wqkv[k][:, m * 128:(m + 1) * 128],
                                xT[k][:, xsl],
                                start=(k == 0), stop=(k == KT - 1))
                for m2 in range(2):
                    m = 0 if m2 == 0 else 2
                    nc.vector.tensor_copy(qk_sb[m][:, csl], psq[m2])

            def rope_chunk(m, c):
                """RoPE on one 512-col chunk of qk_sb[m]: rotate-half via the
                PE permutation matmul, then combine with cos/sin on DVE."""
                csl = slice(c * 512, (c + 1) * 512)
                rp = ps.tile([128, 1024], F32, tag="s", name="rot")
                rps = rp[:, 0:512]
                nc.tensor.matmul(rps, p2t, qk_sb[m][:, csl],
                                 start=True, stop=True)
                tmp = rope_w.tile([128, 512], BF16, tag="rtmp")
                nc.vector.tensor_mul(tmp, rps, sin2[:, csl])
                nc.vector.tensor_mul(qk_sb[m][:, csl], qk_sb[m][:, csl],
                                     cos2[:, csl])
                nc.vector.tensor_add(qk_sb[m][:, csl], qk_sb[m][:, csl], tmp)

            def v_group(tn, cp_act=False):
                mm_ps = ps.tile([128, 1024], F32, tag="s", name="mm_v")
                for k in range(KT):
                    nc.tensor.matmul(
                        mm_ps[:, 0:G * DH],
                        xT[k][:, tn * 128:(tn + 1) * 128],
                        wqkv[k][:, 512:768],
                        start=(k == 0), stop=(k == KT - 1))
                src = mm_ps[:, 0:G * DH].rearrange("p (h d) -> p h d", h=G)
                if cp_act:
                    nc.scalar.copy(v_aug[:, tn, :, 0:DH], src)
                else:
                    nc.vector.tensor_copy(v_aug[:, tn, :, 0:DH], src)
                nc.vector.tensor_copy(v_aug[:, tn, :, DH:DH + 1],
                                      ones_col[:, tn])

            # pair-1 QKV as filler jobs: k-QUARTER per (m2, 512-col chunk) so
            # each job holds a PSUM buf only ~1us; halves are combined via an
            # SBUF staging tile and a DVE add.
            p1_stage = {}

            def pass1_q(m2, chunk, q):
                m = 1 if m2 == 0 else 3
                csl = slice(chunk * 512, (chunk + 1) * 512)
                tt = ps.tile([128, 1024], F32, tag="s", name="p1")
                t = tt[:, 0:512]
                for k in range(q * 4, q * 4 + 4):
                    nc.tensor.matmul(
                        t,
                        wqkv[k][:, m * 128:(m + 1) * 128], xT[k][:, csl],
                        start=(k == q * 4), stop=(k == q * 4 + 3))
                if q == 0:
                    st = rope_w.tile([128, 512], F32, tag="p1st")
                    p1_stage[(m2, chunk)] = st
                    nc.vector.tensor_copy(st, t)
                else:
                    st = p1_stage.pop((m2, chunk))
                    nc.vector.tensor_add(qk_sb[m][:, csl], st, t)

            def normalize(p, iq, o_ps):
                """PV accumulators -> normalized bf16 rows of outT."""
                isl = slice(iq * 512, iq * 512 + 512)
                for hh in range(2):
                    o_sb = norm_w.tile([DH + 1, 512], F32, tag=f"osb{hh}",
                                       name=f"osb{hh}")
                    nc.vector.tensor_copy(o_sb, o_ps[hh])
                    recip0 = norm_w.tile([1, 512], F32, tag=f"r0{hh}",
                                         name=f"r0{hh}")
                    nc.sync.dma_start(out=recip0, in_=o_sb[DH:DH + 1, :])
                    nc.vector.reciprocal_approx_fast(recip0, recip0)
                    bc = norm_w.tile([DH, 512], F32, tag=f"bc{hh}",
                                     name=f"bc{hh}")
                    nc.gpsimd.partition_broadcast(bc, recip0)
                    if hh == 0:
                        nc.vector.tensor_mul(outT[p][0:DH, isl],
                                             o_sb[0:DH, :], bc)
                    else:
                        tmpb = norm_w.tile([DH, 512], BF16, tag="tmpb")
                        nc.vector.tensor_mul(tmpb, o_sb[0:DH, :], bc)
                        nc.sync.dma_start(out=outT[p][DH:2 * DH, isl],
                                          in_=tmpb)

            def emit_pv(p, o_ps, jj, exps):
                for hh in range(2):
                    for half in range(2):
                        j = 2 * jj + half
                        nc.tensor.matmul(
                            o_ps[hh],
                            v_aug[:, j, 2 * p + hh, :],
                            exps[hh][:, half * 512:(half + 1) * 512],
                            start=(j == 0), stop=(j == NT - 1))

            def attention(p, iq, fillers=(), slots=None):
                """Scores -> exp -> PV pipeline for one (pair, i-quarter)
                block; pops filler jobs into PE slack at the given j-slots."""
                o_ps = [pso.tile([DH + 1, 512], F32, tag="o", name=f"o{hh}")
                        for hh in range(2)]
                qT = qk_sb[p]
                kTt = qk_sb[2 + p]
                isl = slice(iq * 512, iq * 512 + 512)
                fillers = list(fillers)
                if slots is None:
                    slots = list(range(len(fillers)))
                fi = 0
                pend_exps = None
                for jj in range(NT // 2):
                    s_ps = [ps.tile([128, 1024], F32, tag="s", name=f"s{hh}")
                            for hh in range(2)]
                    for half in range(2):
                        j = 2 * jj + half
                        jsl = slice(j * 128, (j + 1) * 128)
                        for hh in range(2):
                            hsl = slice(hh * DH, (hh + 1) * DH)
                            # K=64 pair in disjoint PE row groups -> the two
                            # matmuls run concurrently (explicit tile_position;
                            # auto-derive does not engage it)
                            nc.tensor.matmul(
                                s_ps[hh][:, half * 512:(half + 1) * 512],
                                kTt[hsl, jsl], qT[hsl, isl],
                                start=True, stop=True,
                                tile_position=(hh * DH, 0))
                    exps = []
                    for hh in range(2):
                        expT = att.tile([128, 1024], BF16, tag="exp")
                        nc.scalar.activation(expT, s_ps[hh], EXP, scale=SCALE)
                        exps.append(expT)
                    if pend_exps is not None:
                        emit_pv(p, o_ps, jj - 1, pend_exps)
                    pend_exps = exps
                    while fi < len(fillers) and fi < len(slots) \
                            and slots[fi] <= jj:
                        fillers[fi]()
                        fi += 1
                emit_pv(p, o_ps, NT // 2 - 1, pend_exps)
                while fi < len(fillers):
                    fillers[fi]()
                    fi += 1
                normalize(p, iq, o_ps)

            def proj_tile(tn, cp_act=False):
                nsl = slice(tn * 128, (tn + 1) * 128)
                out_sb = outp.tile([128, DIM], BF16, tag="osb")
                f_ps = ps.tile([128, 1024], F32, tag="s", name="f_ps")
                for c2 in range(2):
                    c2sl = slice(c2 * 512, (c2 + 1) * 512)
                    for kk in range(2):
                        nc.tensor.matmul(
                            f_ps[:, c2sl],
                            outT[kk][:, nsl], wout_sb[kk][:, c2sl],
                            start=(kk == 0), stop=(kk == 1))
                if cp_act:
                    nc.scalar.copy(out_sb, f_ps)
                else:
                    nc.vector.tensor_copy(out_sb, f_ps)
                nc.sync.dma_start(
                    out=part_d.ap().rearrange("(t p) m -> t p m", p=128)[tn],
                    in_=out_sb)

            # ---- emission order ----
            # pass0 chunk B needs xT cols 1024:2048, which land ~5us after
            # the first half; v_group(0..7) only reads cols 0:1024, so it
            # fills the PE while that DMA streams.
            qk_pass0(0)
            for m in (0, 2):
                for c in (0, 1):
                    rope_chunk(m, c)
            for tn in range(0, 8):
                v_group(tn, cp_act=True)
            qk_pass0(1)
            for m in (0, 2):
                for c in (2, 3):
                    rope_chunk(m, c)

            def F(fn, *a):
                return lambda: fn(*a)

            # filler inventory for the later attention blocks:
            #   8 v groups, 16 pass-1 k-quarters, 8 pair-1 rope chunks,
            #   12 projection tiles.
            # All pair-1 work must complete inside the pair-0 blocks —
            # attention(1,*) reads the roped qk_sb[1]/[3].  proj(iq) lags its
            # attention blocks by >= 2 block-slots (late slots of block iq+1)
            # so it never waits on a normalize chain.
            f_v = [F(v_group, tn) for tn in range(8, 16)]
            f_p1 = [F(pass1_q, m2, c, q)
                    for m2 in range(2) for c in range(4) for q in range(2)]
            f_r1 = [F(rope_chunk, m, c) for m in (1, 3) for c in range(4)]

            attention(0, 0, f_v)
            attention(0, 1, f_p1[0:8])
            attention(0, 2, f_p1[8:16])
            attention(0, 3, f_r1)
            attention(1, 0)
            attention(1, 1, [F(proj_tile, tn) for tn in range(0, 4)],
                      slots=[4, 5, 6, 7])
            attention(1, 2, [F(proj_tile, tn) for tn in range(4, 8)],
                      slots=[4, 5, 6, 7])
            attention(1, 3, [F(proj_tile, tn) for tn in range(8, 10)],
                      slots=[5, 7])
            # proj 10,11 depend only on normalize(1,2): they keep the PE warm
            # while normalize(1,3) drains; 12-15 follow it.
            for tn in range(10, 12):
                proj_tile(tn)
            for tn in range(12, 16):
                proj_tile(tn, cp_act=(tn % 2 == 0))
    nc.compile()
    _cache["nc"] = nc
    return nc


def kernel(x, w_qkv, w_out, b_out, _trace=False):
    import ml_dtypes
    from concourse.bass_utils import run_bass_kernel_spmd

    x = np.asarray(x, dtype=np.float32)
    w_qkv = np.asarray(w_qkv, dtype=np.float32)
    w_out = np.asarray(w_out, dtype=np.float32)
    b_out = np.asarray(b_out, dtype=np.float32)

    cos2, sin2 = _rope_tables()
    p2t = _p2t()

    in_maps = []
    for c in range(N_CORES):
        b, g = divmod(c, G)
        cols = []
        for blk in range(2):                      # q block, k block
            base = blk * H * DH + g * G * DH
            cols.append(w_qkv[:, base:base + G * DH])
        cols.append(w_qkv[:, 2 * H * DH + g * G * DH:
                          2 * H * DH + (g + 1) * G * DH])   # v block
        wqkv_c = np.ascontiguousarray(np.concatenate(cols, axis=1))  # [DIM,768]
        wout_c = np.ascontiguousarray(
            w_out[g * G * DH:(g + 1) * G * DH, :]).astype(ml_dtypes.bfloat16)
        in_maps.append({
            "xT": np.ascontiguousarray(x[b].T).astype(ml_dtypes.bfloat16),
            "wqkv": wqkv_c.astype(ml_dtypes.bfloat16),
            "wout": wout_c,
            "cos2": cos2.astype(ml_dtypes.bfloat16),
            "sin2": sin2.astype(ml_dtypes.bfloat16),
            "p2t": p2t.astype(ml_dtypes.bfloat16),
        })

    nc = _build()
    res = run_bass_kernel_spmd(nc, in_maps, core_ids=list(range(N_CORES)),
                               trace=_trace)
    out = np.empty((B, N, DIM), dtype=np.float32)
    for b in range(B):
        acc = res.results[G * b]["part"].astype(np.float32)
        for g in range(1, G):
            acc += res.results[G * b + g]["part"].astype(np.float32)
        out[b] = acc + b_out
    if _trace:
        kernel.last_results = res
    return out
